# revision 11
# baseline (speedup 1.0000x reference)
"""Distributed Trainium2 (8 NeuronCores) kernel for masked multi-head attention
+ output projection (nn_Attention_60790967107825).

Strategy (head-parallel attention, row-parallel projection):
  - Each core owns 2 of the 16 heads (all 4 batches) -> 8 (b,h) pairs/core.
  - Scores are computed TRANSPOSED (S^T[j, i] = K Q^T): key mask becomes a
    per-partition ACT bias folded into the exp; softmax denominators come for
    free from an appended ones-column on V; P^T feeds the PV matmul as the
    moving operand with V stationary, so no on-device transposes at all
    (q/k are fed pre-transposed per-head from the host).
  - Unnormalized numerators are scaled by m_i * 1/Z (DVE fast reciprocal +
    DMA partition-broadcast), giving exact masked-query zeroing; the
    uniform-attention (masked query) contribution is rank-1 per batch and is
    re-added AFTER the projection from a V-mean column carried through the
    collective.
  - One AllToAll (~2.1MB/rank) re-shards from head-parallel to row-parallel;
    each core then computes the full 1024-deep projection for its own 1024
    output rows at identical local addresses (SPMD-clean).
"""

import os
import sys

import numpy as np

for _p in ("/opt/trn_rl_repo", "/root/.axon_site/_ro/trn_rl_repo"):
    if os.path.isdir(_p) and _p not in sys.path:
        sys.path.insert(0, _p)

import ml_dtypes  # noqa: E402
import concourse.bass as bass  # noqa: E402,F401
import concourse.mybir as mybir  # noqa: E402
import concourse.tile as tile  # noqa: E402
from concourse import bacc  # noqa: E402
from concourse.bass_utils import run_bass_kernel_spmd  # noqa: E402

B, H, N, D = 4, 16, 2048, 64
DIM = H * D
P = 128
NCORES = 8
HPC = H // NCORES          # heads per core
PAIRS = B * HPC            # (b, h_local) pairs per core
SCALE = float(D) ** -0.5
JT = N // P                # 16 key tiles
IC = 2                     # query chunks per pair
ICW = N // IC              # 1024
RB = B * N // NCORES       # 1024 output rows per core
RBW = RB + 16              # a2a row width (col RB carries the V-mean)
CT = DIM // P              # 8 contraction tiles in the projection
NEGB = -30000.0            # exp bias for masked keys -> exp == 0
MBIG = 1.0e30              # Z multiplier for masked queries -> 1/Z == 0

bf16 = mybir.dt.bfloat16
f32 = mybir.dt.float32
npbf = ml_dtypes.bfloat16

_CACHE = {}


def build_graph():
    nc = bacc.Bacc("TRN2", num_devices=NCORES)

    qT = nc.dram_tensor("qT", [PAIRS, D, N], bf16, kind="ExternalInput")
    kT = nc.dram_tensor("kT", [PAIRS, D, N], bf16, kind="ExternalInput")
    vv = nc.dram_tensor("v", [PAIRS, N, D], bf16, kind="ExternalInput")
    biasD = nc.dram_tensor("biasP", [B, P, JT], f32, kind="ExternalInput")
    minvD = nc.dram_tensor("minv", [PAIRS, N], f32, kind="ExternalInput")
    uD = nc.dram_tensor("uproj", [P, RB // P], f32, kind="ExternalInput")
    wTD = nc.dram_tensor("wT", [DIM, DIM], bf16, kind="ExternalInput")
    boutD = nc.dram_tensor("bout", [1, DIM], f32, kind="ExternalInput")
    outD = nc.dram_tensor("out", [RB, DIM], f32, kind="ExternalOutput")

    with tile.TileContext(nc, num_cores=NCORES) as tc:
        with tc.tile_pool(name="dram", bufs=1, space="DRAM") as dramp:
            a2a_in = dramp.tile([NCORES, P, RBW], bf16, name="a2a_in")
            a2a_out = dramp.tile([NCORES, P, RBW], bf16, name="a2a_out")
            zrow_dram = dramp.tile([PAIRS, N], bf16, name="zrow_dram")
            pvm_dram = dramp.tile([1, DIM], f32, name="pvm_dram")

            with tc.tile_pool(name="constp", bufs=1) as constp:
                bias_sb = constp.tile([P, B, JT], f32, name="bias_sb")
                nc.sync.dma_start(bias_sb[:], biasD.rearrange("b p t -> p b t"))
                ones_col = constp.tile([P, 1], bf16, name="ones_col")
                nc.any.memset(ones_col[:], 1.0)

                with (
                    tc.tile_pool(name="qkp", bufs=2) as qkp,
                    tc.tile_pool(name="vpool", bufs=2) as vp,
                    tc.tile_pool(name="ptp", bufs=3) as ptp,
                    tc.tile_pool(name="onump", bufs=2) as onp,
                    tc.tile_pool(name="smallp", bufs=2) as smallp,
                    tc.tile_pool(name="finp", bufs=2) as finp,
                    tc.tile_pool(name="psS", bufs=2, space="PSUM") as psS,
                    tc.tile_pool(name="psO", bufs=1, space="PSUM") as psO,
                    tc.tile_pool(name="psV", bufs=1, space="PSUM") as psV,
                ):
                    for pr in range(PAIRS):
                        b, hl = divmod(pr, HPC)
                        qt = qkp.tile([P, N], bf16, tag="qt", name=f"qt{pr}")
                        kt = qkp.tile([P, N], bf16, tag="kt", name=f"kt{pr}")
                        nc.any.memset(qt[D:, :], 0.0)
                        nc.any.memset(kt[D:, :], 0.0)
                        nc.sync.dma_start(qt[:D, :], qT[pr])
                        nc.sync.dma_start(kt[:D, :], kT[pr])
                        vt = vp.tile([P, JT, D + 1], bf16, tag="vt", name=f"vt{pr}")
                        nc.any.memset(vt[:, :, D:], 1.0)
                        nc.sync.dma_start(
                            vt[:, :, :D], vv[pr].rearrange("(t pp) d -> pp t d", pp=P)
                        )
                        onum = onp.tile([D, N], bf16, tag="onum", name=f"onum{pr}")
                        vm_ps = psV.tile([D + 1, 1], f32, tag="vm", name=f"vm{pr}")
                        vm16 = smallp.tile([D, 1], bf16, tag="vm16", name=f"vm16{pr}")
                        zpair = smallp.tile([1, N], f32, tag="zpair", name=f"zp{pr}")
                        minv_p = smallp.tile([1, N], f32, tag="minvp", name=f"mi{pr}")
                        nc.sync.dma_start(minv_p[:], minvD[pr : pr + 1, :])

                        for ic in range(IC):
                            i0 = ic * ICW
                            o_ps = psO.tile(
                                [D + 1, ICW], f32, tag="ops", name=f"ops{pr}_{ic}"
                            )
                            for jt in range(JT):
                                s_ps = psS.tile(
                                    [P, ICW], f32, tag="sps", name=f"sps{pr}_{ic}_{jt}"
                                )
                                for n0 in range(0, ICW, 512):
                                    nc.tensor.matmul(
                                        s_ps[:, n0 : n0 + 512],
                                        lhsT=kt[:, jt * P : (jt + 1) * P],
                                        rhs=qt[:, i0 + n0 : i0 + n0 + 512],
                                        start=True,
                                        stop=True,
                                    )
                                pt = ptp.tile(
                                    [P, ICW], bf16, tag="pt", name=f"pt{pr}_{ic}_{jt}"
                                )
                                nc.scalar.activation(
                                    pt[:],
                                    s_ps[:],
                                    mybir.ActivationFunctionType.Exp,
                                    bias=bias_sb[:, b, jt : jt + 1],
                                    scale=SCALE,
                                )
                                for n0 in range(0, ICW, 512):
                                    nc.tensor.matmul(
                                        o_ps[:, n0 : n0 + 512],
                                        lhsT=vt[:, jt, :],
                                        rhs=pt[:, n0 : n0 + 512],
                                        start=(jt == 0),
                                        stop=(jt == JT - 1),
                                    )
                                if ic == 0:
                                    nc.tensor.matmul(
                                        vm_ps[:],
                                        lhsT=vt[:, jt, :],
                                        rhs=ones_col[:],
                                        start=(jt == 0),
                                        stop=(jt == JT - 1),
                                    )
                            # evacuate PSUM quickly: numerators + denominator row
                            nc.vector.tensor_copy(onum[:, i0 : i0 + ICW], o_ps[:D, :])
                            nc.vector.tensor_copy(
                                zpair[0:1, i0 : i0 + ICW], o_ps[D : D + 1, :]
                            )
                        nc.vector.tensor_copy(vm16[:], vm_ps[:D, :])

                        # z path: zm = m_i / Z_i  (masked queries -> 0)
                        zq = smallp.tile([1, N], f32, tag="zq", name=f"zq{pr}")
                        nc.vector.tensor_tensor(
                            zq[:], zpair[:], minv_p[:], mybir.AluOpType.mult
                        )
                        zr = smallp.tile([1, N], f32, tag="zr", name=f"zr{pr}")
                        nc.vector.reciprocal_approx_fast(zr[:], zq[:])
                        zrb = smallp.tile([1, N], bf16, tag="zrb", name=f"zrb{pr}")
                        nc.vector.tensor_copy(zrb[:], zr[:])
                        nc.sync.dma_start(zrow_dram[pr : pr + 1, :], zrb[:])
                        zm64 = finp.tile([D, N], bf16, tag="zm64", name=f"zm64{pr}")
                        nc.sync.dma_start(
                            zm64[:], zrow_dram[pr : pr + 1, :].to_broadcast((D, N))
                        )
                        fin = finp.tile([D, N], bf16, tag="fin", name=f"fin{pr}")
                        nc.vector.tensor_tensor(
                            fin[:], onum[:], zm64[:], mybir.AluOpType.mult
                        )
                        for ic in range(IC):
                            i0 = ic * ICW
                            nc.sync.dma_start(
                                a2a_in[2 * b + ic, hl * D : (hl + 1) * D, 0:RB],
                                fin[:, i0 : i0 + ICW],
                            )
                            nc.sync.dma_start(
                                a2a_in[2 * b + ic, hl * D : (hl + 1) * D, RB : RB + 1],
                                vm16[:],
                            )

            nc.gpsimd.collective_compute(
                "AllToAll",
                mybir.AluOpType.bypass,
                replica_groups=[list(range(NCORES))],
                ins=[a2a_in.opt()],
                outs=[a2a_out.opt()],
            )

            with (
                tc.tile_pool(name="projsb", bufs=1) as prjp,
                tc.tile_pool(name="outp", bufs=3) as outp,
                tc.tile_pool(name="psP", bufs=3, space="PSUM") as psP,
                tc.tile_pool(name="psPV", bufs=1, space="PSUM") as psPV,
            ):
                gat = prjp.tile([P, NCORES, RBW], bf16, name="gat")
                nc.sync.dma_start(gat[:], a2a_out.rearrange("c p r -> p c r"))
                wt_sb = prjp.tile([P, CT, DIM], bf16, name="wt_sb")
                nc.sync.dma_start(
                    wt_sb[:], wTD.rearrange("(ct pp) o -> pp ct o", pp=P)
                )
                u_sb = prjp.tile([P, RB // P], f32, name="u_sb")
                nc.sync.dma_start(u_sb[:], uD[:])
                bout128 = prjp.tile([P, DIM], f32, name="bout128")
                nc.sync.dma_start(bout128[:], boutD[0:1, :].to_broadcast((P, DIM)))

                # projected V-mean row (uniform-attention output before bias)
                pvm_ps = psPV.tile([1, DIM], f32, name="pvm_ps")
                for ct in range(CT):
                    for n0 in range(0, DIM, 512):
                        nc.tensor.matmul(
                            pvm_ps[:, n0 : n0 + 512],
                            lhsT=gat[:, ct, RB : RB + 1],
                            rhs=wt_sb[:, ct, n0 : n0 + 512],
                            start=(ct == 0),
                            stop=(ct == CT - 1),
                        )
                pvm_row = prjp.tile([1, DIM], f32, name="pvm_row")
                nc.vector.tensor_copy(pvm_row[:], pvm_ps[:])
                nc.sync.dma_start(pvm_dram[:], pvm_row[:])
                pvm128 = prjp.tile([P, DIM], f32, name="pvm128")
                nc.sync.dma_start(pvm128[:], pvm_dram[0:1, :].to_broadcast((P, DIM)))

                for rt in range(RB // P):
                    o_ps = psP.tile([P, DIM], f32, tag="prps", name=f"prps{rt}")
                    for ct in range(CT):
                        for n0 in range(0, DIM, 512):
                            nc.tensor.matmul(
                                o_ps[:, n0 : n0 + 512],
                                lhsT=gat[:, ct, rt * P : (rt + 1) * P],
                                rhs=wt_sb[:, ct, n0 : n0 + 512],
                                start=(ct == 0),
                                stop=(ct == CT - 1),
                            )
                    t1 = outp.tile([P, DIM], f32, tag="t1", name=f"t1_{rt}")
                    nc.vector.scalar_tensor_tensor(
                        t1[:],
                        in0=pvm128[:],
                        scalar=u_sb[:, rt : rt + 1],
                        in1=bout128[:],
                        op0=mybir.AluOpType.mult,
                        op1=mybir.AluOpType.add,
                    )
                    osb = outp.tile([P, DIM], f32, tag="osb", name=f"osb{rt}")
                    nc.vector.tensor_tensor(
                        osb[:], o_ps[:], t1[:], mybir.AluOpType.add
                    )
                    nc.sync.dma_start(outD[rt * P : (rt + 1) * P, :], osb[:])

    nc.compile()
    return nc


def _get_nc():
    if "nc" not in _CACHE:
        _CACHE["nc"] = build_graph()
    return _CACHE["nc"]


def make_in_maps(q, k, v, mask, W_out, b_out):
    q16 = q.astype(npbf)
    k16 = k.astype(npbf)
    v16 = v.astype(npbf)
    m_full = np.concatenate(
        [np.ones((B, 1), dtype=bool), np.asarray(mask).astype(bool)], axis=1
    )  # [B, N]
    biasP = (
        np.where(m_full, 0.0, NEGB)
        .astype(np.float32)
        .reshape(B, JT, P)
        .transpose(0, 2, 1)
        .copy()
    )
    minv = np.where(m_full, 1.0, MBIG).astype(np.float32)[
        np.repeat(np.arange(B), HPC)
    ]  # [PAIRS, N]
    wT16 = np.ascontiguousarray(np.asarray(W_out).T).astype(npbf)
    bout = np.asarray(b_out).astype(np.float32).reshape(1, DIM)

    in_maps = []
    for c in range(NCORES):
        heads = slice(HPC * c, HPC * (c + 1))
        qTc = np.ascontiguousarray(
            q16[:, heads].transpose(0, 1, 3, 2).reshape(PAIRS, D, N)
        )
        kTc = np.ascontiguousarray(
            k16[:, heads].transpose(0, 1, 3, 2).reshape(PAIRS, D, N)
        )
        vc = np.ascontiguousarray(v16[:, heads].reshape(PAIRS, N, D))
        bc = c // 2
        i0c = (c % 2) * RB
        u_core = np.ascontiguousarray(
            ((1.0 - m_full[bc, i0c : i0c + RB].astype(np.float32)) / N)
            .reshape(RB // P, P)
            .T
        )
        in_maps.append(
            {
                "qT": qTc,
                "kT": kTc,
                "v": vc,
                "biasP": biasP,
                "minv": minv,
                "uproj": u_core,
                "wT": wT16,
                "bout": bout,
            }
        )
    return in_maps


def run(q, k, v, mask, W_out, b_out, trace=False, **spmd_kwargs):
    nc = _get_nc()
    in_maps = make_in_maps(q, k, v, mask, W_out, b_out)
    res = run_bass_kernel_spmd(
        nc, in_maps, core_ids=list(range(NCORES)), trace=trace, **spmd_kwargs
    )
    outs = [np.asarray(res.results[c]["out"]) for c in range(NCORES)]
    full = np.concatenate(outs, axis=0).reshape(B, N, DIM).astype(np.float32)
    return full, res


def kernel(q, k, v, mask, W_out, b_out):
    out, _ = run(q, k, v, mask, W_out, b_out, trace=False)
    return out
